# revision 1
# baseline (speedup 1.0000x reference)
"""Gaussian-kernel attention (out = x + alpha * exp(-r_sigma*d2(x_i,x_j)) @ x)
for B=4, T=4096, C=64 on 8 trn2 NeuronCores.

Sharding: core = b*2 + h handles batch b, query rows [h*2048, (h+1)*2048).
Each core receives x[b] ROTATED so its own query rows come first
(xrot = roll(x[b], -h*2048, axis=0)); key order is a permutation (the sum
over keys is permutation-invariant, so results are unchanged).  The host
also stages xrot^T (f32) so the kernel needs no on-device transposes.

The kernel exp factorizes:  K = exp(-r*d2) = es_s * E * w_t  with
  E[s,t] = exp(2r * <x_s, x_t>)   (the only T x T term)
  es_s   = exp(-r*|x_s|^2)        folded into the stage-2 weights
  w_t    = exp(-r*|x_t|^2)        folded into the epilogue
so the hot loop is:
  stage 1:  S = A^T R, contraction 64:  A = x^T, R = (2r*x)^T  (bf16).
            Each 128-key chunk is split into two 64-key col-tiled
            matmuls on the two PE column groups: their LDWEIGHTS are
            column-disjoint, so weight loads hide under streaming and
            both halves stream concurrently (the PE is hard-capped at
            1.2 GHz here; tile concurrency is the only way past
            1 column/cycle).
  exp:      E = exp(S) on ScalarE, immediate scale/bias (AP operands
            cost +330ns/instr), 1024-wide from PSUM.
  stage 2:  P[tb*64+c, q] = (alpha*es*x_chunk)^T @ E_chunk, the two
            query halves col-tiled into PSUM partition groups so one
            (128,1024) partial serves both; DVE accumulates partials
            into an SBUF f32 accumulator (frees all 8 PSUM banks for
            a 4-slot s_ps/partial pipeline).
  epilogue: res = x^T + acc * W  (W = w_t broadcast via tiny matmuls).

At the actual operating point (r_sigma = 0): R = 0 exactly, S = 0, E = 1,
es = w = 1 exactly, so only the single bf16 rounding of x in stage 2
matters (~3e-4 scale-relative output error).  The reference's clamp of
d2 at 0 only suppresses ~1e-6-scale rounding noise and is skipped.
"""

import numpy as np

B, T, C = 4, 4096, 64
NCORES = 8
ROWS = T // 2        # query rows per core
TB = 1024            # t-block width (one exp tile; 2 PSUM banks)
SC = 128             # s-chunk (keys per inner step)
NSC = T // SC        # 32
NTB = ROWS // TB     # 2
MMN = 512            # max matmul free dim (one PSUM bank of f32)

_CACHE = {}


def _build_program():
    from contextlib import ExitStack

    import concourse.bass as bass  # noqa: F401
    import concourse.mybir as mybir
    import concourse.tile as tile
    from concourse import bacc

    f32 = mybir.dt.float32
    bf16 = mybir.dt.bfloat16
    Exp = mybir.ActivationFunctionType.Exp

    nc = bacc.Bacc(None, target_bir_lowering=False)
    xf = nc.dram_tensor("xf", (128, NSC * C), f32, kind="ExternalInput")
    xtf = nc.dram_tensor("xtf", (C, T), f32, kind="ExternalInput")
    rsig = nc.dram_tensor("rsig", (1, 1), f32, kind="ExternalInput")
    alp = nc.dram_tensor("alp", (1, 1), f32, kind="ExternalInput")
    out = nc.dram_tensor("out_ct", (2 * C, TB), f32, kind="ExternalOutput")

    with ExitStack() as ctx:
        tc = ctx.enter_context(tile.TileContext(nc))
        cp = ctx.enter_context(tc.tile_pool(name="const", bufs=1))

        # ---- input loads ----
        # scalars first: DMA-completion sems are FIFO per queue, so tiny
        # loads must not queue behind the 2MB bulk transfers
        rsig_sb = cp.tile([1, 1], f32)
        nc.sync.dma_start(rsig_sb[:], rsig[:])
        alp_sb = cp.tile([1, 1], f32)
        nc.sync.dma_start(alp_sb[:], alp[:])
        xtfd = cp.tile([64, T], f32)      # x^T exact (gates stage-1: first)
        nc.sync.dma_start(xtfd[:, 0:ROWS], xtf[:, 0:ROWS])
        nc.sync.dma_start(xtfd[:, ROWS:T], xtf[:, ROWS:T])
        # xf arrives host-chunked: xf[p, si*C + c] = x[si*128 + p, c]
        xf_sb = cp.tile([128, NSC * C], f32)
        nc.sync.dma_start(xf_sb[:, 0:NSC * C // 2], xf[:, 0:NSC * C // 2])
        nc.sync.dma_start(xf_sb[:, NSC * C // 2:], xf[:, NSC * C // 2:])

        ones_row = cp.tile([1, 128], f32)
        nc.vector.memset(ones_row, 1.0)
        tl_out = cp.tile([1, 1], f32)
        ones_rb = cp.tile([1, 128], bf16)
        nc.vector.memset(ones_rb, 1.0)
        ones_c64 = cp.tile([64, 1], bf16)
        nc.vector.memset(ones_c64, 1.0)

        # ---- derived operands ----
        A_big = cp.tile([64, T], bf16)       # x^T
        R_big = cp.tile([64, ROWS], bf16)    # 2r * x^T
        xa_sb = cp.tile([128, NSC * C], bf16)   # alpha*es*x (stage-2 weights)
        xsqn = cp.tile([128, NSC * C], f32)  # x*x natural layout
        sqn = cp.tile([128, NSC], f32)       # |x_s|^2 per key
        nrsq = cp.tile([128, NSC], f32)      # -r*|x_s|^2
        es_sb = cp.tile([128, NSC], f32)     # exp(-r*|x_s|^2)
        aes_sb = cp.tile([128, NSC], f32)    # alpha * es
        xsqT = cp.tile([64, ROWS], bf16)     # (x^T)^2 for w_t
        nsqT = cp.tile([1, ROWS], f32)       # -r*|x_t|^2
        wexp = cp.tile([1, ROWS], bf16)      # w_t = exp(-r*|x_t|^2)
        rb_sb = cp.tile([128, 1], f32)       # r broadcast
        rb2_sb = cp.tile([128, 1], f32)      # 2r broadcast
        nrb_sb = cp.tile([128, 1], f32)      # -r broadcast
        ab_sb = cp.tile([128, 1], f32)       # alpha broadcast

        with tc.tile_pool(name="pre", bufs=1, space="PSUM") as pre:
            # broadcast scalars across partitions (tiny matmuls)
            rb_ps = pre.tile([128, 1], f32)
            nc.tensor.matmul(rb_ps, ones_row, rsig_sb[:], start=True, stop=True)
            al_ps = pre.tile([128, 1], f32)
            nc.tensor.matmul(al_ps, ones_row, alp_sb[:], start=True, stop=True)
            # pre-load the Exp table set during the input DMAs
            nc.scalar.activation(tl_out, ones_row[0:1, 0:1], Exp)
            # A = x^T cast and R = 2r*x^T on DVE (idle at the head; keep
            # the ACT stream clear -- it is the kernel bottleneck), chunked
            # so they start as the xtfd DMA halves land
            for g in range(4):
                gs = slice(g * (T // 4), (g + 1) * (T // 4))
                nc.vector.tensor_copy(A_big[:, gs], xtfd[:, gs])
            nc.vector.tensor_scalar_mul(rb2_sb, rb_ps, 2.0)
            nc.vector.tensor_scalar_mul(nrb_sb, rb_ps, -1.0)
            for g in range(2):
                gs = slice(g * TB, (g + 1) * TB)
                nc.vector.tensor_scalar_mul(R_big[:, gs], xtfd[:, gs],
                                            rb2_sb[0:64, :])
            nc.vector.tensor_copy(rb_sb, rb_ps)
            nc.vector.tensor_copy(ab_sb, al_ps)

        # ---- main loop: col-tiled stage 1 AND stage 2 ----
        with (
            tc.tile_pool(name="spool", bufs=3, space="PSUM") as spool,
            tc.tile_pool(name="opool", bufs=1, space="PSUM") as opool,
            tc.tile_pool(name="kpool", bufs=6) as kpool,
        ):
            # persistent PSUM accumulator: [0:64]=queries tb0, [64:128]=tb1
            ot = opool.tile([128, TB], f32)
            pending = None
            for p in range(NSC // 2):
                c0, c1 = 2 * p, 2 * p + 1
                x0 = slice(c0 * C, (c0 + 1) * C)
                x1 = slice(c1 * C, (c1 + 1) * C)
                if p > 0:
                    # stage-2 weights for this pair (DVE, off critical path)
                    nc.vector.tensor_scalar_mul(
                        xa_sb[:, x0], xf_sb[:, x0], aes_sb[:, c0:c0 + 1])
                    nc.vector.tensor_scalar_mul(
                        xa_sb[:, x1], xf_sb[:, x1], aes_sb[:, c1:c1 + 1])
                if p == 10:
                    # t-side squares for the epilogue (DVE idle mid-loop)
                    for h in range(ROWS // MMN):
                        hs = slice(h * MMN, (h + 1) * MMN)
                        nc.vector.tensor_mul(xsqT[:, hs], xtfd[:, hs],
                                             xtfd[:, hs])
                for j in range(NTB):
                    tba, tbb = j, 1 - j
                    kk = {}
                    for ci, tb in ((c0, tba), (c1, tbb)):
                        s_ps = spool.tile([128, TB], f32, tag="s_ps")
                        # key chunk split into two 64-key halves on the two
                        # PE column groups; their LDWEIGHTS are column-
                        # disjoint so load/stream overlap and both halves
                        # stream concurrently.
                        for h in range(TB // MMN):
                            hs = slice(h * MMN, (h + 1) * MMN)
                            qs = slice(tb * TB + h * MMN,
                                       tb * TB + (h + 1) * MMN)
                            for g in range(2):
                                asl = slice(ci * SC + 64 * g,
                                            ci * SC + 64 * (g + 1))
                                nc.tensor.matmul(
                                    s_ps[64 * g:64 * (g + 1), hs],
                                    A_big[:, asl], R_big[:, qs],
                                    start=True, stop=True)
                        k_sb = kpool.tile([128, TB], bf16, tag="k")
                        nc.scalar.activation(k_sb, s_ps, Exp)
                        kk[tb] = (k_sb, ci)
                    if p == 0 and j == 0:
                        # s-side chain, after the first two exps are queued:
                        # |x_s|^2 -> -r*sq -> es -> aes -> first xa weights
                        nc.vector.tensor_mul(xsqn, xf_sb, xf_sb)
                        nc.vector.tensor_reduce(
                            sqn, xsqn.rearrange("p (n c) -> p n c", c=C),
                            axis=mybir.AxisListType.X, op=mybir.AluOpType.add,
                        )
                        nc.vector.tensor_scalar_mul(nrsq, sqn, nrb_sb)
                        nc.scalar.activation(es_sb, nrsq, Exp)
                        nc.vector.tensor_scalar_mul(aes_sb, es_sb, ab_sb)
                        nc.vector.tensor_scalar_mul(
                            xa_sb[:, x0], xf_sb[:, x0], aes_sb[:, c0:c0 + 1])
                        nc.vector.tensor_scalar_mul(
                            xa_sb[:, x1], xf_sb[:, x1], aes_sb[:, c1:c1 + 1])
                    # stage 2 is emitted one iteration late: by then both
                    # exp tiles are done, so these matmuls never stall the
                    # in-order PE queue ahead of the next stage-1 fills.
                    if pending is not None:
                        pk, ptba, ptbb, pfirst = pending
                        for h in range(TB // MMN):
                            hs = slice(h * MMN, (h + 1) * MMN)
                            for tb in (0, 1):
                                pk_sb, pci = pk[tb]
                                xs = slice(pci * C, (pci + 1) * C)
                                nc.tensor.matmul(
                                    ot[64 * tb:64 * tb + 64, hs],
                                    xa_sb[:, xs], pk_sb[:, hs],
                                    start=pfirst, stop=False)
                    pending = (kk, tba, tbb, p == 0 and j == 0)

            # last deferred stage-2 group closes the accumulation
            pk, ptba, ptbb, pfirst = pending
            for h in range(TB // MMN):
                hs = slice(h * MMN, (h + 1) * MMN)
                for tb in (0, 1):
                    pk_sb, pci = pk[tb]
                    xs = slice(pci * C, (pci + 1) * C)
                    nc.tensor.matmul(ot[64 * tb:64 * tb + 64, hs],
                                     xa_sb[:, xs], pk_sb[:, hs],
                                     start=pfirst, stop=True)

            # t-side: |x_t|^2 via ones-matmul partition reduce -> w_t
            for h in range(ROWS // MMN):
                hs = slice(h * MMN, (h + 1) * MMN)
                sq_ps = spool.tile([1, MMN], f32, tag="s_ps")
                nc.tensor.matmul(sq_ps[:], ones_c64, xsqT[:, hs],
                                 start=True, stop=True)
                nc.vector.tensor_scalar_mul(nsqT[0:1, hs], sq_ps[:],
                                            nrb_sb[0:1, :])

            # ---- epilogue: res = x^T + ot * W  (W = w_t broadcast) ----
            nsqT_bf = cp.tile([1, ROWS], bf16)
            nc.vector.tensor_copy(nsqT_bf, nsqT)
            W_ps = spool.tile([128, TB], f32, tag="s_ps")
            for g in range(2):
                for h in range(TB // MMN):
                    hs = slice(h * MMN, (h + 1) * MMN)
                    ws = slice(g * TB + h * MMN, g * TB + (h + 1) * MMN)
                    nc.tensor.matmul(W_ps[64 * g:64 * g + 64, hs],
                                     ones_rb[:, 0:64], nsqT_bf[0:1, ws],
                                     start=True, stop=True)
            W_sb = cp.tile([128, TB], bf16)
            nc.scalar.activation(W_sb, W_ps, Exp)
            exT = cp.tile([128, TB], f32)   # x^T packed as [tb*64+c, q]
            nc.sync.dma_start(exT[0:64, :], xtf[:, 0:TB])
            nc.sync.dma_start(exT[64:128, :], xtf[:, TB:ROWS])
            res = cp.tile([128, TB], f32)
            nc.vector.tensor_mul(res, ot, W_sb)
            nc.vector.tensor_add(res, res, exT)
            nc.sync.dma_start(out[:], res[:])

    return nc


def _get_program():
    if "nc" not in _CACHE:
        nc = _build_program()
        if not nc.is_finalized():
            nc.finalize()  # runs Bacc legalization (wait splitting, reg alloc)
        _CACHE["nc"] = nc
    return _CACHE["nc"]


def _make_in_maps(x, r_sigma, alpha):
    x = np.asarray(x, np.float32)
    rs = np.float32(np.asarray(r_sigma).reshape(())).reshape(1, 1)
    al = np.float32(np.asarray(alpha).reshape(())).reshape(1, 1)
    in_maps = []
    for core in range(NCORES):
        b, h = divmod(core, 2)
        xrot = np.roll(x[b], -h * ROWS, axis=0)
        xfc = xrot.reshape(NSC, SC, C).transpose(1, 0, 2).reshape(SC, NSC * C)
        in_maps.append({
            "xf": np.ascontiguousarray(xfc),
            "xtf": np.ascontiguousarray(xrot.T),
            "rsig": np.ascontiguousarray(rs),
            "alp": np.ascontiguousarray(al),
        })
    return in_maps


def kernel_with_results(x, r_sigma, alpha, trace=False):
    from concourse.bass_utils import run_bass_kernel_spmd

    nc = _get_program()
    res = run_bass_kernel_spmd(
        nc, _make_in_maps(x, r_sigma, alpha), core_ids=list(range(NCORES)),
        trace=trace,
    )
    out = np.empty((B, T, C), np.float32)
    for core in range(NCORES):
        b, h = divmod(core, 2)
        r = res.results[core]["out_ct"].reshape(NTB, C, TB)
        out[b, h * ROWS:(h + 1) * ROWS] = (
            r.transpose(0, 2, 1).reshape(ROWS, C)
        )
    return out, res


def kernel(x, r_sigma, alpha):
    out, _ = kernel_with_results(x, r_sigma, alpha)
    return out

